# revision 8
# baseline (speedup 1.0000x reference)
"""DMLSTM Trainium2 kernel (B=64, S=2048, I=H=512), data-parallel over batch
across 8 NeuronCores.

Strategy (per core, batch shard BL=8):
- Everything lives in a transposed ("feature-major") layout: z.T tiles
  [128 units, 8 batch] so ACT/DVE run at full partition width.
- Recurrent matmul z_h.T = W_h.T @ h.T with W_h tiles as the stationary
  operand (bf16 -> fast weight load), h.T [128, 8] as the moving operand.
- Input projection Zx = x @ W_x + b is precomputed per 64-step chunk as a
  fat matmul (moving N=512) and double-buffered in SBUF.
- The 80 (m, kc) matmuls of each recurrent step are statically scheduled so
  that each output chunk's point-products finish early enough for the gate
  math (ACT sigmoid/tanh + DVE cell update) to produce h.T chunks before the
  next step's matmuls need them -> PE never stalls.
- Sequence loop: For_i over 16 bodies x 2 chunks (ping-pong buffers),
  dynamic DRAM slices for x/y chunk DMAs.

Output y is stored bf16 in [chunk, kc, p, t*8+b] layout; the host reassembles
and upcasts to fp32.
"""

import numpy as np
import ml_dtypes

import concourse.bass as bass
import concourse.tile as tile
import concourse.mybir as mybir
from concourse import bacc
from concourse.bass import ds
from concourse.bass_utils import run_bass_kernel_spmd

BF16 = ml_dtypes.bfloat16
F32 = mybir.dt.float32
BF = mybir.dt.bfloat16

N_CORES = 8
B, S, I, H = 64, 2048, 512, 512
BL = B // N_CORES          # batch per core = 8
KC = 4                     # contraction chunks (512/128) for each of W_x, W_h
NG = 5                     # gates i,f,o,c,d
NM = KC * NG               # 20 m-tiles of 128 output units
FG = 5 * H                 # 2560


def mm_schedule(kc_min=(0, 0, 16, 34)):
    """Static order of the 80 (m, kc) recurrent matmuls within one step.

    kc_min[kc] = earliest position at which a matmul contracting h-chunk kc may
    run (h chunk kc of the PREVIOUS step must have cleared the gate pipeline).
    Greedy: always progress the lowest-index output chunk (m // NG) so its
    gates can start early.
    """
    rem = [(m, kc) for m in range(NM) for kc in range(KC)]
    order = []
    for q in range(NM * KC):
        elig = [mk for mk in rem if q >= kc_min[mk[1]]]
        assert elig, f"infeasible schedule at {q}"
        elig.sort(key=lambda mk: (mk[0] // NG, mk[1], mk[0]))
        pick = elig[0]
        rem.remove(pick)
        order.append(pick)
    # completion position of each output chunk
    comp = [max(i for i, (m, kc) in enumerate(order) if m // NG == k)
            for k in range(KC)]
    return order, comp


def build_nc(s=S, tc_steps=64, kc_min=(0, 0, 16, 34)):
    """Build the SPMD Bass program (same program on all 8 cores)."""
    nch = s // tc_steps          # chunks
    assert nch % 2 == 0 and nch >= 2
    nbody = nch // 2
    TC = tc_steps
    ydt = BF
    order, comp = mm_schedule(kc_min)
    # PSUM zero-region = one 2KB bank = one z_ps[k] tile, so each chunk's
    # 20 matmuls form ONE accumulation group: start on the chronologically
    # first matmul into that tile, stop on the last.
    first_q = {}
    for q, (m, kc) in enumerate(order):
        first_q.setdefault(m // NG, q)

    nc = bacc.Bacc("TRN2", target_bir_lowering=False, debug=False,
                   num_devices=N_CORES)

    xT = nc.dram_tensor("xT", [KC, 128, nch + 2, TC * BL], BF,
                        kind="ExternalInput")
    w = nc.dram_tensor("w", [128, 2 * KC, NM, 128], BF, kind="ExternalInput")
    bt = nc.dram_tensor("bt", [128, NM], F32, kind="ExternalInput")
    y = nc.dram_tensor("y", [nch, KC, 128, TC * BL], ydt, kind="ExternalOutput")

    with tile.TileContext(nc) as tc:
        import contextlib
        with contextlib.ExitStack() as ctx:
            singles = ctx.enter_context(tc.tile_pool(name="singles", bufs=1))
            zx_psum = ctx.enter_context(
                tc.tile_pool(name="zx_psum", bufs=2, space="PSUM"))
            z_psum = ctx.enter_context(
                tc.tile_pool(name="z_psum", bufs=1, space="PSUM"))
            gp = ctx.enter_context(tc.tile_pool(name="gates", bufs=3))

            # ---- persistent SBUF ----
            w_sb = singles.tile([128, 2 * KC, NM, 128], BF, name="w_sb", tag="w_sb")
            bt_sb = singles.tile([128, NM], F32, name="bt_sb", tag="bt_sb")
            c_st = singles.tile([128, KC * BL], F32, name="c_st", tag="c_st")
            xb = [singles.tile([128, KC, TC * BL], BF, name=f"xb{p}", tag=f"xb{p}")
                  for p in range(2)]
            zxb = [singles.tile([128, TC, 4 * NG * BL], F32, name=f"zxb{p}", tag=f"zxb{p}")
                   for p in range(2)]
            hb = [singles.tile([128, KC, TC * BL], BF, name=f"hb{p}", tag=f"hb{p}")
                  for p in range(2)]
            # per-output-chunk PSUM accumulators for the recurrent matmul
            z_ps = [z_psum.tile([128, NG * BL], F32, name=f"zps{k}", tag=f"zps{k}")
                    for k in range(KC)]

            # ---- prologue ----
            nc.sync.dma_start(out=w_sb, in_=w.ap())
            nc.sync.dma_start(out=bt_sb, in_=bt.ap())
            nc.vector.memset(c_st, 0.0)
            nc.vector.memset(hb[1], 0.0)
            for kc in range(KC):
                nc.sync.dma_start(out=xb[0][:, kc:kc + 1, :],
                                  in_=xT[kc, :, 0:1, :])
                nc.sync.dma_start(out=xb[1][:, kc:kc + 1, :],
                                  in_=xT[kc, :, 1:2, :])

            def zx_block(xbuf, zxbuf):
                """Zx chunk = (x_chunk @ W_x).T + b -> zxbuf [128, TC, 160]."""
                for m in range(NM):
                    ps = zx_psum.tile([128, TC * BL], F32, name="zx_ps", tag="zx_ps")
                    for kc in range(KC):
                        nc.tensor.matmul(ps, w_sb[:, kc, m, :],
                                         xbuf[:, kc, :],
                                         start=(kc == 0), stop=(kc == KC - 1))
                    nc.scalar.activation(
                        out=zxbuf[:, :, m * BL:(m + 1) * BL],
                        in_=ps.rearrange("p (t b) -> p t b", b=BL),
                        func=mybir.ActivationFunctionType.Identity,
                        bias=bt_sb[:, m:m + 1], scale=1.0)

            def step(t, hbuf, hbuf_prev, zxbuf):
                """One recurrent timestep; h(t-1) read from hbuf (or tail of
                hbuf_prev when t == 0), h(t) written to hbuf."""
                def h_rhs(kc):
                    if t == 0:
                        return hbuf_prev[:, kc, (TC - 1) * BL: TC * BL]
                    return hbuf[:, kc, (t - 1) * BL: t * BL]

                done = 0
                for q, (m, kc) in enumerate(order):
                    k, g = divmod(m, NG)
                    nc.tensor.matmul(
                        z_ps[k][:, g * BL:(g + 1) * BL],
                        w_sb[:, KC + kc, m, :], h_rhs(kc),
                        start=(q == first_q[k]), stop=(q == comp[k]))
                    while done < KC and q == comp[done]:
                        gates(t, done, hbuf, zxbuf)
                        done += 1

            def gates(t, k, hbuf, zxbuf):
                cs = c_st[:, k * BL:(k + 1) * BL]
                z = gp.tile([128, NG * BL], F32, name="z", tag="z")
                nc.vector.tensor_add(z, z_ps[k],
                                     zxbuf[:, t, k * NG * BL:(k + 1) * NG * BL])
                sg = gp.tile([128, 3 * BL], F32, name="sg", tag="sg")
                nc.scalar.activation(out=sg, in_=z[:, 0:3 * BL],
                                     func=mybir.ActivationFunctionType.Sigmoid)
                th = gp.tile([128, 2 * BL], F32, name="th", tag="th")
                nc.scalar.activation(out=th, in_=z[:, 3 * BL:5 * BL],
                                     func=mybir.ActivationFunctionType.Tanh)
                ig = gp.tile([128, BL], F32, name="ig", tag="ig")
                nc.vector.tensor_mul(ig, sg[:, 0:BL], th[:, 0:BL])       # i*g
                fc = gp.tile([128, BL], F32, name="fc", tag="fc")
                nc.vector.tensor_mul(fc, sg[:, BL:2 * BL], cs)           # f*c
                dc = gp.tile([128, BL], F32, name="dc", tag="dc")
                nc.vector.tensor_mul(dc, th[:, BL:2 * BL], cs)           # d*c
                cp = gp.tile([128, BL], F32, name="cp", tag="cp")
                nc.vector.tensor_add(cp, ig, fc)                         # c'
                u = gp.tile([128, BL], F32, name="u", tag="u")
                nc.vector.scalar_tensor_tensor(                          # (1+d)*c'
                    out=u, in0=th[:, BL:2 * BL], scalar=1.0, in1=cp,
                    op0=mybir.AluOpType.add, op1=mybir.AluOpType.mult)
                nc.vector.tensor_sub(cs, u, dc)                          # c_new
                tc_t = gp.tile([128, BL], F32, name="tc_t", tag="tc_t")
                nc.scalar.activation(out=tc_t, in_=cs,
                                     func=mybir.ActivationFunctionType.Tanh)
                hsl = hbuf[:, k, t * BL:(t + 1) * BL]
                nc.vector.tensor_mul(hsl, sg[:, 2 * BL:3 * BL], tc_t)    # h

            # Zx(chunk 0) into zxb[0]
            zx_block(xb[0], zxb[0])

            def body(j):
                c0 = 2 * j            # even chunk
                # 1) prefetch x(c0+2) into xb[0]
                for kc in range(KC):
                    nc.sync.dma_start(out=xb[0][:, kc:kc + 1, :],
                                      in_=xT[kc, :, ds(c0 + 2, 1), :])
                # 2) Zx(c0+1) -> zxb[1]
                zx_block(xb[1], zxb[1])
                # 3) steps of chunk c0 (reads zxb[0], h ping: hb[0])
                for t in range(TC):
                    step(t, hb[0], hb[1], zxb[0])
                for kc in range(KC):
                    nc.sync.dma_start(out=y[ds(c0, 1), kc, :, :],
                                      in_=hb[0][:, kc:kc + 1, :])
                # 4) Zx(c0+2) -> zxb[0]
                zx_block(xb[0], zxb[0])
                # 5) prefetch x(c0+3) into xb[1]
                for kc in range(KC):
                    nc.sync.dma_start(out=xb[1][:, kc:kc + 1, :],
                                      in_=xT[kc, :, ds(c0 + 3, 1), :])
                # 6) steps of chunk c0+1
                for t in range(TC):
                    step(t, hb[1], hb[0], zxb[1])
                for kc in range(KC):
                    nc.sync.dma_start(out=y[ds(c0 + 1, 1), kc, :, :],
                                      in_=hb[1][:, kc:kc + 1, :])

            with tc.For_i(0, nbody, 1) as j:
                body(j)

    nc.compile()
    return nc


# ---------------- host-side driver ----------------

def _prep_core_inputs(x_core, W, b, tc_steps):
    """x_core [BL, s, I] fp32 -> per-core input map."""
    bl, s, ii = x_core.shape
    nch = s // tc_steps
    # xT [KC, 128, nch+2, TC*BL]: xT[kc, p, c, t*BL+b] = x[b, c*TC+t, kc*128+p]
    xr = x_core.reshape(bl, nch, tc_steps, ii).transpose(3, 1, 2, 0)
    xr = np.ascontiguousarray(xr).reshape(KC, 128, nch, tc_steps * bl)
    xT = np.zeros((KC, 128, nch + 2, tc_steps * bl), dtype=BF16)
    xT[:, :, :nch, :] = xr.astype(BF16)
    return xT


def _prep_weights(W, b):
    """W [1024, 2560] -> w [128, 2*KC, NM, 128] bf16; bt [128, NM] f32.

    m = k_out*NG + g covers W columns [g*512 + k_out*128, +128).
    kci 0..3 -> W_x rows (x part), 4..7 -> W_h rows (h part).
    """
    wt = np.empty((128, 2 * KC, NM, 128), dtype=BF16)
    btm = np.empty((128, NM), dtype=np.float32)
    for kci in range(2 * KC):
        rows = slice(kci * 128, (kci + 1) * 128)
        for m in range(NM):
            k_out, g = divmod(m, NG)
            cols = slice(g * H + k_out * 128, g * H + (k_out + 1) * 128)
            # w[p, kci, m, j] = W[kci*128+p, g*512+k_out*128+j]
            wt[:, kci, m, :] = W[rows, cols].astype(BF16)
    for m in range(NM):
        k_out, g = divmod(m, NG)
        btm[:, m] = b[g * H + k_out * 128: g * H + (k_out + 1) * 128]
    return wt, btm


def _assemble_output(y_cores, s, tc_steps):
    """y_cores: list of [nch, KC, 128, TC*BL] bf16 -> [B, s, H] fp32."""
    nch = s // tc_steps
    out = np.empty((N_CORES * BL, s, H), dtype=np.float32)
    for i, yc in enumerate(y_cores):
        # yc[c, k, p, t*BL + b] = h[b, c*TC+t, k*128+p]
        v = np.asarray(yc).reshape(nch, KC, 128, tc_steps, BL)
        v = v.transpose(4, 0, 3, 1, 2).reshape(BL, s, H)
        out[i * BL:(i + 1) * BL] = v.astype(np.float32)
    return out


_NC_CACHE = {}


def _get_nc(s, tc_steps):
    key = (s, tc_steps)
    if key not in _NC_CACHE:
        _NC_CACHE[key] = build_nc(s=s, tc_steps=tc_steps)
    return _NC_CACHE[key]


def _install_ntff_shim():
    """Best-effort: register the axon NTFF profile hook so trace=True works
    in containers whose antenv lacks axon_hooks. No-op on failure."""
    try:
        import sys
        import types
        import antenv

        if "antenv.axon_hooks" in sys.modules:
            return True
        _hook_box = {}

        def get_axon_ntff_profile_hook():
            return _hook_box.get("h")

        def set_axon_ntff_profile_hook(hook):
            _hook_box["h"] = hook

        mod = types.ModuleType("antenv.axon_hooks")
        mod.get_axon_ntff_profile_hook = get_axon_ntff_profile_hook
        mod.set_axon_ntff_profile_hook = set_axon_ntff_profile_hook
        sys.modules["antenv.axon_hooks"] = mod
        antenv.axon_hooks = mod
        from trn_agent_boot.trn_boot import _ntff_profile_via_ctypes

        set_axon_ntff_profile_hook(
            _ntff_profile_via_ctypes("/opt/axon/libaxon_pjrt.so"))
        return _hook_box.get("h") is not None
    except Exception:
        return False


def kernel(x_enc, W, b, tc_steps=64, trace=False):
    if trace:
        _install_ntff_shim()
    x_enc = np.asarray(x_enc)
    W = np.asarray(W)
    b = np.asarray(b)
    s = x_enc.shape[1]
    nc = _get_nc(s, tc_steps)
    wt, btm = _prep_weights(W, b)
    in_maps = []
    for i in range(N_CORES):
        xT = _prep_core_inputs(x_enc[i * BL:(i + 1) * BL], W, b, tc_steps)
        in_maps.append({"xT": xT, "w": wt, "bt": btm})
    res = run_bass_kernel_spmd(nc, in_maps, core_ids=list(range(N_CORES)),
                               trace=trace)
    y_cores = [res.results[i]["y"] for i in range(N_CORES)]
    out = _assemble_output(y_cores, s, tc_steps)
    if trace:
        kernel.last_results = res
    return out


# revision 10
# speedup vs baseline: 1.0402x; 1.0402x over previous
"""DMLSTM Trainium2 kernel (B=64, S=2048, I=H=512), data-parallel over batch
across 8 NeuronCores.

Per core (batch shard BL=8), everything runs in feature-major layout
(z.T tiles [128 units, 8 batch]):

- Recurrent matmul z_h.T = W_h.T @ h.T: W_h bf16 tiles stationary (fast
  weight load), h.T [128, 8] moving. 80 (m, kc) matmuls per step plus two
  identity matmuls that inject the precomputed input projection Zx into the
  same PSUM accumulation (no separate DVE add).
- All sigmoids are computed as tanh: sigmoid(z) = 0.5 (1 + tanh(z/2)) with
  the 0.5 z-scaling folded into W/b on the host, so the five gates of a
  half-step are ONE ACT tanh over PSUM. The (1+t)/2 affines fold into the
  cell algebra (state q = 2c) and the 2x-scaled hidden state h2 = 2h, whose
  0.5 folds into W_h (host) and into the output scaling (host).
- Cell update per half (chunks of 256 hidden units), all ops 2-input
  DVE tensor ops on [128, 2, 8] views:
      dq = d*q_prev; e2 = (ti+1)*g; f2 = (tf+1)*c; s = e2+f2
      tq = (d+1)*s;  q = tq - dq        (= 2*c_new)
      tanhc = ACT tanh(q, scale=0.5); h2 = (to+1)*tanhc   (bf16)
  plus one c = 0.5*q per step on GPSIMD.
- z PSUM is double buffered by step parity: 4 banks (half x parity).
- The 80+2 matmuls of each step follow a static schedule: half-0 outputs
  complete early so their gate pipeline overlaps the remaining matmuls;
  matmuls contracting h-half-1 are pushed late (kc_min) so the previous
  step's gate chain has time to deliver h2.
- Zx for the next chunk is computed by matmuls sprinkled between steps
  (filling PE stall slots), evacuated PSUM->SBUF bf16 by ACT with the bias
  add folded in.
- Sequence loop: For_i over 16 bodies x 2 chunks (ping-pong buffers),
  dynamic DRAM slices for x/y chunk DMAs.

Output y is h2 = 2h in bf16, layout [chunk, kc, p, t*8+b]; the host
reassembles, scales by 0.5 and upcasts to fp32.
"""

import numpy as np
import ml_dtypes

import concourse.bass as bass
import concourse.tile as tile
import concourse.mybir as mybir
from concourse import bacc
from concourse.bass import ds
from concourse.bass_utils import run_bass_kernel_spmd
from concourse.masks import make_identity

BF16 = ml_dtypes.bfloat16
F32 = mybir.dt.float32
BF = mybir.dt.bfloat16
AF = mybir.ActivationFunctionType
ALU = mybir.AluOpType

N_CORES = 8
B, S, I, H = 64, 2048, 512, 512
BL = B // N_CORES          # batch per core = 8
KC = 4                     # contraction chunks (512/128) for each of W_x, W_h
NG = 5                     # gates i,f,o,c,d
NM = KC * NG               # 20 m-tiles of 128 output units
NH = 2                     # h halves (of 2 chunks each)


def mm_schedule(kc_min23=32):
    """Static order of the 80 (m, kc) recurrent matmuls within one step.

    Matmuls contracting h-chunks 2,3 (previous step's half-1 output) may not
    run before position kc_min23. Greedy otherwise: finish output half 0
    first so its gate pipeline starts early.
    Returns (order, comp_half, first_half) where comp_half[h]/first_half[h]
    are the last/first positions of output-half h.
    """
    rem = [(m, kc) for m in range(NM) for kc in range(KC)]
    order = []
    for q in range(NM * KC):
        elig = [mk for mk in rem if (mk[1] < 2 or q >= kc_min23)]
        assert elig, f"infeasible schedule at {q}"
        elig.sort(key=lambda mk: (mk[0] // 10, mk[1], mk[0]))
        pick = elig[0]
        rem.remove(pick)
        order.append(pick)
    comp_half = [max(i for i, (m, kc) in enumerate(order) if m // 10 == h)
                 for h in range(NH)]
    first_half = [min(i for i, (m, kc) in enumerate(order) if m // 10 == h)
                  for h in range(NH)]
    return order, comp_half, first_half


def build_nc(s=S, tc_steps=64, kc_min23=32):
    """Build the SPMD Bass program (same program on all 8 cores)."""
    nch = s // tc_steps          # chunks
    assert nch % 2 == 0 and nch >= 2
    nbody = nch // 2
    TC = tc_steps
    order, comp_half, first_half = mm_schedule(kc_min23)

    # zx sprinkle: m-tile job of the NEXT chunk emitted after step t
    zx_sched = {}
    for m in range(NM):
        zx_sched.setdefault(min(m * TC // NM, TC - 1), []).append(m)

    nc = bacc.Bacc("TRN2", target_bir_lowering=False, debug=False,
                   num_devices=N_CORES)

    xT = nc.dram_tensor("xT", [KC, 128, nch + 2, TC * BL], BF,
                        kind="ExternalInput")
    w = nc.dram_tensor("w", [128, 2 * KC, NM, 128], BF, kind="ExternalInput")
    bt = nc.dram_tensor("bt", [128, NM], F32, kind="ExternalInput")
    y = nc.dram_tensor("y", [nch, KC, 128, TC * BL], BF, kind="ExternalOutput")

    with tile.TileContext(nc) as tc:
        import contextlib
        with contextlib.ExitStack() as ctx:
            singles = ctx.enter_context(tc.tile_pool(name="singles", bufs=1))
            zx_psum = ctx.enter_context(
                tc.tile_pool(name="zx_psum", bufs=2, space="PSUM"))
            z_psum = ctx.enter_context(
                tc.tile_pool(name="z_psum", bufs=1, space="PSUM"))
            gp = ctx.enter_context(tc.tile_pool(name="gates", bufs=3))

            # ---- persistent SBUF ----
            w_sb = singles.tile([128, 2 * KC, NM, 128], BF, name="w_sb")
            bt_sb = singles.tile([128, NM], F32, name="bt_sb")
            ident = singles.tile([128, 128], BF, name="ident")
            c_st = singles.tile([128, KC, BL], F32, name="c_st")
            q_st = singles.tile([128, KC, BL], F32, name="q_st")
            xb = [singles.tile([128, KC, TC * BL], BF, name=f"xb{p}")
                  for p in range(2)]
            zxb = [singles.tile([128, TC, NM * BL], BF, name=f"zxb{p}")
                   for p in range(2)]
            hb = [singles.tile([128, KC, TC * BL], BF, name=f"hb{p}")
                  for p in range(2)]
            # z PSUM: one bank per (half, parity)
            z_ps = [[z_psum.tile([128, 2 * NG * BL], F32, name=f"zps{h}_{p}")
                     for p in range(2)] for h in range(NH)]

            # ---- prologue ----
            nc.sync.dma_start(out=w_sb, in_=w.ap())
            nc.sync.dma_start(out=bt_sb, in_=bt.ap())
            make_identity(nc, ident)
            nc.vector.memset(c_st, 0.0)
            nc.vector.memset(q_st, 0.0)
            nc.vector.memset(hb[1], 0.0)
            for kc in range(KC):
                nc.sync.dma_start(out=xb[0][:, kc:kc + 1, :],
                                  in_=xT[kc, :, 0:1, :])
                nc.sync.dma_start(out=xb[1][:, kc:kc + 1, :],
                                  in_=xT[kc, :, 1:2, :])

            def zx_job(m, xbuf, zxbuf):
                """One m-tile of the next chunk's input projection."""
                ps = zx_psum.tile([128, TC * BL], F32, name="zx_ps",
                                  tag="zx_ps")
                for kc in range(KC):
                    nc.tensor.matmul(ps, w_sb[:, kc, m, :], xbuf[:, kc, :],
                                     start=(kc == 0), stop=(kc == KC - 1))
                nc.scalar.activation(
                    out=zxbuf[:, :, m * BL:(m + 1) * BL],
                    in_=ps.rearrange("p (t b) -> p t b", b=BL),
                    func=AF.Identity, bias=bt_sb[:, m:m + 1], scale=1.0)

            def gates(t, h, hbuf, zp):
                """Gate + cell math for output half h (hidden chunks 2h,2h+1)."""
                cslc = slice(2 * h, 2 * h + 2)
                thz = gp.tile([128, 2 * NG * BL], F32, name="thz", tag="thz")
                nc.scalar.activation(out=thz, in_=zp, func=AF.Tanh)
                tv = thz.rearrange("p (c g b) -> p c g b", g=NG, b=BL)
                ti, tf, to = tv[:, :, 0, :], tv[:, :, 1, :], tv[:, :, 2, :]
                g_, d_ = tv[:, :, 3, :], tv[:, :, 4, :]
                qh = q_st[:, cslc, :]
                ch = c_st[:, cslc, :]
                dq = gp.tile([128, 2, BL], F32, name="dq", tag="dq")
                nc.vector.tensor_mul(dq, d_, qh)                    # d*q_prev
                e2 = gp.tile([128, 2, BL], F32, name="e2", tag="e2")
                nc.vector.scalar_tensor_tensor(
                    out=e2, in0=ti, scalar=1.0, in1=g_,
                    op0=ALU.add, op1=ALU.mult)                      # (1+ti)*g
                f2 = gp.tile([128, 2, BL], F32, name="f2", tag="f2")
                nc.vector.scalar_tensor_tensor(
                    out=f2, in0=tf, scalar=1.0, in1=ch,
                    op0=ALU.add, op1=ALU.mult)                      # (1+tf)*c
                s_ = gp.tile([128, 2, BL], F32, name="s_", tag="s_")
                nc.vector.tensor_add(s_, e2, f2)                    # = 2cp
                tq = gp.tile([128, 2, BL], F32, name="tq", tag="tq")
                nc.vector.scalar_tensor_tensor(
                    out=tq, in0=d_, scalar=1.0, in1=s_,
                    op0=ALU.add, op1=ALU.mult)                      # (1+d)*2cp
                nc.vector.tensor_sub(qh, tq, dq)                    # q = 2c_new
                thc = gp.tile([128, 2 * BL], F32, name="thc", tag="thc")
                nc.scalar.activation(out=thc,
                                     in_=qh.rearrange("p c b -> p (c b)"),
                                     func=AF.Tanh, scale=0.5)
                hsl = hbuf[:, cslc, t * BL:(t + 1) * BL]
                nc.vector.scalar_tensor_tensor(
                    out=hsl, in0=to, scalar=1.0,
                    in1=thc.rearrange("p (c b) -> p c b", b=BL),
                    op0=ALU.add, op1=ALU.mult)                      # h2 = 2h

            def step(t, hbuf, hbuf_prev, zxbuf, zx_src, zx_dst):
                par = t % 2

                def h_rhs(kc):
                    if t == 0:
                        return hbuf_prev[:, kc, (TC - 1) * BL: TC * BL]
                    return hbuf[:, kc, (t - 1) * BL: t * BL]

                # identity matmuls inject zx (start each half's psum group)
                def ident_mm(h):
                    nc.tensor.matmul(
                        z_ps[h][par], ident,
                        zxbuf[:, t, h * 10 * BL:(h + 1) * 10 * BL],
                        start=True, stop=False)

                ident_mm(0)
                ident_mm(1)
                for q, (m, kc) in enumerate(order):
                    h, ml = divmod(m, 10)
                    nc.tensor.matmul(
                        z_ps[h][par][:, ml * BL:(ml + 1) * BL],
                        w_sb[:, KC + kc, m, :], h_rhs(kc),
                        start=False, stop=(q == comp_half[h]))
                    if q == comp_half[0]:
                        gates(t, 0, hbuf, z_ps[0][par])
                if True:
                    gates(t, 1, hbuf, z_ps[1][par])
                # c = q/2 for the next step's f2 (off the critical chain)
                nc.gpsimd.tensor_scalar_mul(c_st, q_st, 0.5)
                for m in zx_sched.get(t, ()):
                    zx_job(m, zx_src, zx_dst)

            def chunk_steps(hbuf, hbuf_prev, zxbuf, zx_src, zx_dst):
                for t in range(TC):
                    step(t, hbuf, hbuf_prev, zxbuf, zx_src, zx_dst)

            # Zx(chunk 0) monolithic
            for m in range(NM):
                zx_job(m, xb[0], zxb[0])

            def body(j):
                c0 = 2 * j            # even chunk
                for kc in range(KC):
                    nc.sync.dma_start(out=xb[0][:, kc:kc + 1, :],
                                      in_=xT[kc, :, ds(c0 + 2, 1), :])
                # chunk A = c0: consumes zxb[0], sprinkles Zx(c0+1) -> zxb[1]
                chunk_steps(hb[0], hb[1], zxb[0], xb[1], zxb[1])
                for kc in range(KC):
                    nc.sync.dma_start(out=y[ds(c0, 1), kc, :, :],
                                      in_=hb[0][:, kc:kc + 1, :])
                for kc in range(KC):
                    nc.sync.dma_start(out=xb[1][:, kc:kc + 1, :],
                                      in_=xT[kc, :, ds(c0 + 3, 1), :])
                # chunk B = c0+1: consumes zxb[1], sprinkles Zx(c0+2) -> zxb[0]
                chunk_steps(hb[1], hb[0], zxb[1], xb[0], zxb[0])
                for kc in range(KC):
                    nc.sync.dma_start(out=y[ds(c0 + 1, 1), kc, :, :],
                                      in_=hb[1][:, kc:kc + 1, :])

            with tc.For_i(0, nbody, 1) as j:
                body(j)

    nc.compile()
    return nc


# ---------------- host-side driver ----------------

def _prep_core_inputs(x_core, tc_steps):
    """x_core [BL, s, I] fp32 -> xT [KC, 128, nch+2, TC*BL] bf16."""
    bl, s, ii = x_core.shape
    nch = s // tc_steps
    xr = x_core.reshape(bl, nch, tc_steps, ii).transpose(3, 1, 2, 0)
    xr = np.ascontiguousarray(xr).reshape(KC, 128, nch, tc_steps * bl)
    xT = np.zeros((KC, 128, nch + 2, tc_steps * bl), dtype=BF16)
    xT[:, :, :nch, :] = xr.astype(BF16)
    return xT


def _prep_weights(W, b):
    """W [1024, 2560] -> w [128, 2*KC, NM, 128] bf16; bt [128, NM] f32.

    m = k_out*NG + g covers W columns [g*512 + k_out*128, +128).
    kci 0..3 -> W_x rows, 4..7 -> W_h rows.
    Scaling: sigmoid gates (i,f,o = g 0..2) become tanh(z/2): scale their
    W columns and b by 0.5. The h2 = 2h state folds another 0.5 into all
    W_h columns.
    """
    col_scale = np.ones(5 * H, np.float32)
    col_scale[:3 * H] = 0.5
    Ws = W * col_scale[None, :]
    Ws[I:] *= 0.5
    bs = b * col_scale
    wt = np.empty((128, 2 * KC, NM, 128), dtype=BF16)
    btm = np.empty((128, NM), dtype=np.float32)
    for kci in range(2 * KC):
        rows = slice(kci * 128, (kci + 1) * 128)
        for m in range(NM):
            k_out, g = divmod(m, NG)
            cols = slice(g * H + k_out * 128, g * H + (k_out + 1) * 128)
            wt[:, kci, m, :] = Ws[rows, cols].astype(BF16)
    for m in range(NM):
        k_out, g = divmod(m, NG)
        btm[:, m] = bs[g * H + k_out * 128: g * H + (k_out + 1) * 128]
    return wt, btm


def _assemble_output(y_cores, s, tc_steps):
    """y_cores: list of [nch, KC, 128, TC*BL] bf16 (h2=2h) -> [B,s,H] fp32."""
    nch = s // tc_steps
    out = np.empty((N_CORES * BL, s, H), dtype=np.float32)
    for i, yc in enumerate(y_cores):
        v = np.asarray(yc).reshape(nch, KC, 128, tc_steps, BL)
        v = v.transpose(4, 0, 3, 1, 2).reshape(BL, s, H)
        out[i * BL:(i + 1) * BL] = v.astype(np.float32)
    out *= 0.5
    return out


_NC_CACHE = {}


def _get_nc(s, tc_steps):
    key = (s, tc_steps)
    if key not in _NC_CACHE:
        _NC_CACHE[key] = build_nc(s=s, tc_steps=tc_steps)
    return _NC_CACHE[key]


def _install_ntff_shim():
    """Best-effort: register the axon NTFF profile hook so trace=True works
    in containers whose antenv lacks axon_hooks. No-op on failure."""
    try:
        import sys
        import types
        import antenv

        if "antenv.axon_hooks" in sys.modules:
            return True
        _hook_box = {}

        def get_axon_ntff_profile_hook():
            return _hook_box.get("h")

        def set_axon_ntff_profile_hook(hook):
            _hook_box["h"] = hook

        mod = types.ModuleType("antenv.axon_hooks")
        mod.get_axon_ntff_profile_hook = get_axon_ntff_profile_hook
        mod.set_axon_ntff_profile_hook = set_axon_ntff_profile_hook
        sys.modules["antenv.axon_hooks"] = mod
        antenv.axon_hooks = mod
        from trn_agent_boot.trn_boot import _ntff_profile_via_ctypes

        set_axon_ntff_profile_hook(
            _ntff_profile_via_ctypes("/opt/axon/libaxon_pjrt.so"))
        return _hook_box.get("h") is not None
    except Exception:
        return False


def kernel(x_enc, W, b, tc_steps=64, trace=False):
    if trace:
        _install_ntff_shim()
    x_enc = np.asarray(x_enc)
    W = np.asarray(W)
    b = np.asarray(b)
    s = x_enc.shape[1]
    nc = _get_nc(s, tc_steps)
    wt, btm = _prep_weights(W, b)
    in_maps = []
    for i in range(N_CORES):
        xT = _prep_core_inputs(x_enc[i * BL:(i + 1) * BL], tc_steps)
        in_maps.append({"xT": xT, "w": wt, "bt": btm})
    res = run_bass_kernel_spmd(nc, in_maps, core_ids=list(range(N_CORES)),
                               trace=trace)
    y_cores = [res.results[i]["y"] for i in range(N_CORES)]
    out = _assemble_output(y_cores, s, tc_steps)
    if trace:
        kernel.last_results = res
    return out
